# revision 2
# baseline (speedup 1.0000x reference)
"""v7: row-flat layout, all-static DMAs, 2-dim DRAM access patterns.

Problem: x [64, 3, 512, 512] f32, shifts [64, 2] int32 in [-16, 16].
out[b, c, h, w] = x[b, c, (h - shifts[b,0]) % 512, (w - shifts[b,1]) % 512]

Pure data parallel over batch (8 per core). Per batch:
- H-roll applied on the DRAM->SBUF load through a 33-case If chain (sync/SP
  engine). Every load box has a fully CONTIGUOUS DRAM side (consecutive rows
  of one channel), so nothing gets compiler-unrolled.
- W-roll applied on the SBUF->DRAM store through a 33-case If chain (scalar/
  ACT engine, its own HWDGE ring). The SBUF row-flat layout makes the DRAM
  side a uniform [1536 rows x w-slice] 2-dim pattern - native PDMA2D shape.

SBUF layout per slot: [128 partitions, 12, 512]: global row r = c*512 + h
lives at partition r // 12, free slot r % 12. (Any DMA whose DRAM-side AP
needs >2 dims is statically unrolled ~128x by the compiler - that's what
makes naive layouts take an hour to compile.)

Cross-engine pipelining uses per-slot semaphores (a single counting
semaphore would be ambiguous: batch b+1 completions could satisfy batch b's
wait because DMA completions are unordered).
"""

from contextlib import ExitStack

import numpy as np

import concourse.bass as bass
import concourse.mybir as mybir
from concourse.bass_utils import run_bass_kernel_spmd

B_TOTAL, C, H, W = 64, 3, 512, 512
N_CORES = 8
B = B_TOTAL // N_CORES
MAX_SHIFT = 16
P = 128
J = (C * H) // P  # 12 rows per partition; r = c*512 + h = p*J + j
NBUF = 4

LOADS_PER_BATCH = 18  # semaphore-equalized
STORES_PER_BATCH = 2


def _copy_rows(sync, tile_s, r0, x_src, load_sem):
    """Copy len(src rows) contiguous DRAM rows into tile rows [r0, r0+n).

    x_src: DRAM AP [n, 512] (contiguous rows of one channel).
    Emits 1-3 DMAs (partial head partition / full body / partial tail).
    Returns DMA count.
    """
    n = x_src.shape[0]
    cnt = 0
    lo = r0
    hi = r0 + n
    src = 0
    if lo % J != 0 and lo < hi:
        m = min(hi - lo, J - lo % J)
        p = lo // J
        sync.dma_start(
            tile_s[p : p + 1, (lo % J) * W : (lo % J + m) * W],
            x_src[src : src + m].rearrange("r w -> (r w)")[None, :],
        ).then_inc(load_sem, 16)
        cnt += 1
        lo += m
        src += m
    nfull = (hi - lo) // J
    if nfull > 0:
        # max_dma_last_dim=2048 caps descriptors at 2 KB. The default
        # 24 KB-per-partition descriptors defeat the HWDGE packet
        # round-robin: they all land on SDMA engine 0, serializing the
        # whole load stream at ~27 GB/s (one engine's line rate).
        sync.dma_start(
            tile_s[lo // J : lo // J + nfull, :],
            x_src[src : src + nfull * J].rearrange("(p q) w -> p (q w)", q=J),
            max_dma_last_dim=2048,
        ).then_inc(load_sem, 16)
        cnt += 1
        lo += nfull * J
        src += nfull * J
    if lo < hi:
        m = hi - lo
        p = lo // J
        sync.dma_start(
            tile_s[p : p + 1, 0 : m * W],
            x_src[src : src + m].rearrange("r w -> (r w)")[None, :],
        ).then_inc(load_sem, 16)
        cnt += 1
    return cnt


def _emit_loads(sync, x, tile_s, b, hoff, load_sem):
    """tile row (c*512 + h) = x[b, c, (h + hoff) % 512, :]."""
    n = 0
    if hoff == 0:
        for c in range(C):
            n += _copy_rows(sync, tile_s, c * H, x[b, c, :, :], load_sem)
    else:
        R = H - hoff
        for c in range(C):
            # piece 1: dst rows [c*512, c*512+R) <- src h [hoff, 512)
            n += _copy_rows(sync, tile_s, c * H, x[b, c, hoff:H, :], load_sem)
            # piece 2: dst rows [c*512+R, c*512+512) <- src h [0, hoff)
            n += _copy_rows(sync, tile_s, c * H + R, x[b, c, 0:hoff, :], load_sem)
    assert n <= LOADS_PER_BATCH, (hoff, n)
    if n < LOADS_PER_BATCH:
        sync.sem_inc(load_sem, 16 * (LOADS_PER_BATCH - n))
    return n


def _emit_stores(nc, scalar, out, tile_s, b, woff, store_sem):
    """out[b, c, h, w] = tile[c*512+h, (w + woff) % 512]."""
    out_rw = out[b].rearrange("c h w -> (c h) w")  # [1536, 512] uniform stride
    tile_j = tile_s.rearrange("p (j w) -> p j w", w=W)
    if woff == 0:
        scalar.dma_start(out_rw, tile_s[:, :]).then_inc(store_sem, 16)
        n = 1
    else:
        with nc.allow_non_contiguous_dma(
            reason="W-roll wrap strip can be a single column"
        ):
            # box D: out[.., 0:W-woff] = tile[.., woff:W]
            scalar.dma_start(
                out_rw[:, 0 : W - woff], tile_j[:, :, woff:W]
            ).then_inc(store_sem, 16)
            # box E: out[.., W-woff:W] = tile[.., 0:woff]
            scalar.dma_start(
                out_rw[:, W - woff : W], tile_j[:, :, 0:woff]
            ).then_inc(store_sem, 16)
        n = 2
    if n < STORES_PER_BATCH:
        scalar.sem_inc(store_sem, 16 * (STORES_PER_BATCH - n))
    return n


def build_kernel():
    nc = bass.Bass()
    x = nc.dram_tensor("x", [B, C, H, W], mybir.dt.float32, kind="ExternalInput")
    shifts = nc.dram_tensor("shifts", [B, 2], mybir.dt.int32, kind="ExternalInput")
    out = nc.dram_tensor("out", [B, C, H, W], mybir.dt.float32, kind="ExternalOutput")

    with (
        nc.sbuf_tensor([P, NBUF, J * W], mybir.dt.float32) as tiles,
        nc.sbuf_tensor([1, B * 2], mybir.dt.int32) as sb_shifts,
        nc.semaphore("pre_sem") as pre_sem,
        ExitStack() as stack,
    ):
        load_sems = [
            stack.enter_context(nc.semaphore(f"load_sem{s}")) for s in range(NBUF)
        ]
        store_sems = [
            stack.enter_context(nc.semaphore(f"store_sem{s}")) for s in range(NBUF)
        ]
        block = stack.enter_context(nc.Block())

        @block.sync
        def _(sync):
            sync.dma_start(
                sb_shifts[0:1, :], shifts.rearrange("b s -> (b s)")[None, :]
            ).then_inc(pre_sem, 16)
            sync.wait_ge(pre_sem, 16)
            with sync.register("r_sh") as r_sh:
                for b in range(B):
                    s = b % NBUF
                    if b >= NBUF:
                        sync.wait_ge(
                            store_sems[s], 16 * STORES_PER_BATCH * (b // NBUF)
                        )
                    sync.reg_load(r_sh, sb_shifts[0:1, 2 * b : 2 * b + 1])
                    sh = sync.snap(r_sh)
                    for v in range(-MAX_SHIFT, MAX_SHIFT + 1):
                        with sync.If(sh == v):
                            _emit_loads(
                                sync, x, tiles[:, s], b, (-v) % H, load_sems[s]
                            )

        @block.scalar
        def _(scalar):
            scalar.wait_ge(pre_sem, 16)
            with scalar.register("r_sw") as r_sw:
                for b in range(B):
                    s = b % NBUF
                    scalar.wait_ge(
                        load_sems[s], 16 * LOADS_PER_BATCH * (b // NBUF + 1)
                    )
                    scalar.reg_load(r_sw, sb_shifts[0:1, 2 * b + 1 : 2 * b + 2])
                    sw = scalar.snap(r_sw)
                    for v in range(-MAX_SHIFT, MAX_SHIFT + 1):
                        with scalar.If(sw == v):
                            _emit_stores(
                                nc,
                                scalar,
                                out,
                                tiles[:, s],
                                b,
                                (-v) % W,
                                store_sems[s],
                            )
            for s in range(NBUF):
                uses = (B - s + NBUF - 1) // NBUF
                scalar.wait_ge(store_sems[s], 16 * STORES_PER_BATCH * uses)

    return nc


_NC_CACHE = None


def _get_nc():
    global _NC_CACHE
    if _NC_CACHE is None:
        _NC_CACHE = build_kernel()
    return _NC_CACHE


def kernel(x: np.ndarray, shifts: np.ndarray) -> np.ndarray:
    assert x.shape == (B_TOTAL, C, H, W), x.shape
    assert shifts.shape == (B_TOTAL, 2), shifts.shape
    x = np.ascontiguousarray(x, dtype=np.float32)
    shifts = np.ascontiguousarray(shifts, dtype=np.int32)

    in_maps = [
        {"x": x[i * B : (i + 1) * B], "shifts": shifts[i * B : (i + 1) * B]}
        for i in range(N_CORES)
    ]
    res = run_bass_kernel_spmd(_get_nc(), in_maps, list(range(N_CORES)))
    return np.concatenate(
        [res.results[i]["out"] for i in range(N_CORES)], axis=0
    ).astype(np.float32)



# revision 6
# speedup vs baseline: 5.0862x; 5.0862x over previous
"""v9: circular-padded input + dynamic-offset full-partition DMAs.

Problem: x [64, 3, 512, 512] f32, shifts [64, 2] int32 in [-16, 16].
out[b, c, h, w] = x[b, c, (h - shifts[b,0]) % 512, (w - shifts[b,1]) % 512]

Pure data parallel over batch (8 per core). Host-side, each channel is
circular-padded by MAX_SHIFT=16 on every border (544x544) and stored as
one flat row of a [24, 296000] tensor (64 tail-pad elems so the widest
dynamic window stays in bounds). BOTH rolls then become a plain window
read at element offset (16-sh)*544 + (16-sw) -- no wraparound pieces,
no If chains. Each (batch, channel) is ONE fixed-shape [128, 4, 512]
load DMA whose DRAM offset is a single register value; the store is a
fully static contiguous DMA.

Why [128, ...] everywhere: the HWDGE fans one DMA's descriptors across
n SDMA engines where n = largest divisor <= 16 of the partition count
(measured: 128 parts -> 16 engines at 341 GB/s; 42 -> 14; 41 (prime) ->
~1 engine at 24 GB/s). The previous design's H-roll pieces had 36-43
partition bodies -- a gcd lottery that serialized ~70% of load bytes on
one engine (761 us total vs ~141 us HBM roofline).

Engine assignment: each dynamic-offset DMA permanently consumes a few
sequencer registers at trace time (offset + bounds-check lowering), and
an engine has only 49 -- 24 dynamic loads on one engine exhausts the
file. So the dynamic loads are split across the two HWDGE sequencers
(sync: batches 0-3, scalar: batches 4-7, own register files) and the
static stores go through gpsimd (SWDGE, measured 377 GB/s on
[128, x, 512] shapes). Offsets are computed with in-place reg ALU ops
on two reused registers (reg_alu with an int immediate also leaks a
const register per call, so 16 and 544 are hoisted into registers).

SBUF layout per slot: [128, 3, 4, 512]: tile[p, c*2048 + j*512 + w]
holds out[b, c, p*4 + j, w].
"""

from contextlib import ExitStack

import numpy as np

import concourse.bass as bass
import concourse.mybir as mybir
from bass_rust import RegisterHandles, make_scalar_value
from concourse.bass_utils import run_bass_kernel_spmd

B_TOTAL, C, H, W = 64, 3, 512, 512
N_CORES = 8
B = B_TOTAL // N_CORES
MAX_SHIFT = 16
PAD = 2 * MAX_SHIFT  # 32
HP, WP = H + PAD, W + PAD  # 544, 544
CH_ELEMS = HP * WP  # 295936
CH_STRIDE = CH_ELEMS + 64  # 296000, tail pad keeps max window in bounds
P = 128
JH = H // P  # 4 rows of a channel per partition
WIN = P * JH * WP  # 278528: window covering 512 padded rows
MAX_OFF = PAD * WP + PAD  # 17440
NBUF = 6

LOADS_PER_BATCH = C
STORES_PER_BATCH = C


def build_kernel():
    nc = bass.Bass()
    x = nc.dram_tensor("x", [B * C, CH_STRIDE], mybir.dt.float32, kind="ExternalInput")
    shifts = nc.dram_tensor("shifts", [B, 2], mybir.dt.int32, kind="ExternalInput")
    out = nc.dram_tensor("out", [B, C, H, W], mybir.dt.float32, kind="ExternalOutput")

    CW = JH * W  # 2048 elems per channel per partition

    with (
        nc.sbuf_tensor([P, NBUF, C * CW], mybir.dt.float32) as tiles,
        nc.sbuf_tensor([1, B * 2], mybir.dt.int32) as sb_shifts,
        nc.semaphore("pre_sem") as pre_sem,
        ExitStack() as stack,
    ):
        load_sems = [
            stack.enter_context(nc.semaphore(f"load_sem{s}")) for s in range(NBUF)
        ]
        store_sems = [
            stack.enter_context(nc.semaphore(f"store_sem{s}")) for s in range(NBUF)
        ]
        block = stack.enter_context(nc.Block())

        def emit_loads(eng, my_batches):
            with (
                eng.register("r_off") as r_off,
                eng.register("r_sw") as r_sw,
                eng.register("r_c16") as r_c16,
                eng.register("r_cWP") as r_cWP,
            ):
                eng.reg_mov(r_c16, MAX_SHIFT)
                eng.reg_mov(r_cWP, WP)
                for b in my_batches:
                    s = b % NBUF
                    if b >= NBUF:
                        eng.wait_ge(
                            store_sems[s], 16 * STORES_PER_BATCH * (b // NBUF)
                        )
                    eng.reg_load(r_off, sb_shifts[0:1, 2 * b : 2 * b + 1])
                    eng.reg_load(r_sw, sb_shifts[0:1, 2 * b + 1 : 2 * b + 2])
                    # r_off = (16 - sh) * 544 + (16 - sw)
                    eng.reg_sub(r_off, r_c16, r_off)
                    eng.reg_sub(r_sw, r_c16, r_sw)
                    eng.reg_mul(r_off, r_off, r_cWP)
                    eng.reg_add(r_off, r_off, r_sw)
                    rb = make_scalar_value(
                        RegisterHandles([r_off]), min_val=0, max_val=MAX_OFF
                    )
                    tile_s = tiles[:, s]
                    for c in range(C):
                        win = x[b * C + c, bass.ds(rb, WIN)]
                        src = win.rearrange("(p j w) -> p j w", j=JH, w=WP)[
                            :, :, 0:W
                        ]
                        eng.dma_start(
                            tile_s[:, c * CW : (c + 1) * CW], src
                        ).then_inc(load_sems[s], 16)

        @block.sync
        def _(sync):
            sync.dma_start(
                sb_shifts[0:1, :], shifts.rearrange("b s -> (b s)")[None, :]
            ).then_inc(pre_sem, 16)
            sync.wait_ge(pre_sem, 16)
            emit_loads(sync, list(range(0, B // 2)))

        @block.scalar
        def _(scalar):
            scalar.wait_ge(pre_sem, 16)
            emit_loads(scalar, list(range(B // 2, B)))

        @block.gpsimd
        def _(gp):
            for b in range(B):
                s = b % NBUF
                gp.wait_ge(load_sems[s], 16 * LOADS_PER_BATCH * (b // NBUF + 1))
                tile_s = tiles[:, s]
                for c in range(C):
                    gp.dma_start(
                        out[b, c].rearrange("(p j) w -> p (j w)", j=JH),
                        tile_s[:, c * CW : (c + 1) * CW],
                    ).then_inc(store_sems[s], 16)
            for s in range(NBUF):
                uses = (B - s + NBUF - 1) // NBUF
                gp.wait_ge(store_sems[s], 16 * STORES_PER_BATCH * uses)

    return nc


_NC_CACHE = None


def _get_nc():
    global _NC_CACHE
    if _NC_CACHE is None:
        _NC_CACHE = build_kernel()
    return _NC_CACHE


def _pad_input(x: np.ndarray) -> np.ndarray:
    """[64, 3, 512, 512] -> [64*3, 296000]: per-channel circular 16-px
    border (544x544) flattened, with 64 tail-pad elems per channel."""
    xp = np.pad(
        x,
        ((0, 0), (0, 0), (MAX_SHIFT, MAX_SHIFT), (MAX_SHIFT, MAX_SHIFT)),
        mode="wrap",
    ).reshape(B_TOTAL * C, CH_ELEMS)
    outp = np.zeros((B_TOTAL * C, CH_STRIDE), dtype=np.float32)
    outp[:, :CH_ELEMS] = xp
    return outp


def kernel(x: np.ndarray, shifts: np.ndarray) -> np.ndarray:
    assert x.shape == (B_TOTAL, C, H, W), x.shape
    assert shifts.shape == (B_TOTAL, 2), shifts.shape
    x = np.ascontiguousarray(x, dtype=np.float32)
    shifts = np.ascontiguousarray(shifts, dtype=np.int32)
    x_pad = _pad_input(x)

    in_maps = [
        {
            "x": x_pad[i * B * C : (i + 1) * B * C],
            "shifts": shifts[i * B : (i + 1) * B],
        }
        for i in range(N_CORES)
    ]
    res = run_bass_kernel_spmd(_get_nc(), in_maps, list(range(N_CORES)))
    return np.concatenate(
        [res.results[i]["out"] for i in range(N_CORES)], axis=0
    ).astype(np.float32)


# revision 7
# speedup vs baseline: 5.1999x; 1.0224x over previous
"""v9: circular-padded input + dynamic-offset full-partition DMAs.

Problem: x [64, 3, 512, 512] f32, shifts [64, 2] int32 in [-16, 16].
out[b, c, h, w] = x[b, c, (h - shifts[b,0]) % 512, (w - shifts[b,1]) % 512]

Pure data parallel over batch (8 per core). Host-side, each channel is
circular-padded by MAX_SHIFT=16 on every border (544x544) and stored as
one flat row of a [24, 296000] tensor (64 tail-pad elems so the widest
dynamic window stays in bounds). BOTH rolls then become a plain window
read at element offset (16-sh)*544 + (16-sw) -- no wraparound pieces,
no If chains. Each (batch, channel) is ONE fixed-shape [128, 4, 512]
load DMA whose DRAM offset is a single register value; the store is a
fully static contiguous DMA.

Why [128, ...] everywhere: the HWDGE fans one DMA's descriptors across
n SDMA engines where n = largest divisor <= 16 of the partition count
(measured: 128 parts -> 16 engines at 341 GB/s; 42 -> 14; 41 (prime) ->
~1 engine at 24 GB/s). The previous design's H-roll pieces had 36-43
partition bodies -- a gcd lottery that serialized ~70% of load bytes on
one engine (761 us total vs ~141 us HBM roofline).

Engine assignment: each dynamic-offset DMA permanently consumes a few
sequencer registers at trace time (offset + bounds-check lowering), and
an engine has only 49 -- 24 dynamic loads on one engine exhausts the
file. So the dynamic loads are split across the two HWDGE sequencers
(sync: batches 0-3, scalar: batches 4-7, own register files) and the
static stores go through gpsimd (SWDGE, measured 377 GB/s on
[128, x, 512] shapes). Offsets are computed with in-place reg ALU ops
on two reused registers (reg_alu with an int immediate also leaks a
const register per call, so 16 and 544 are hoisted into registers).

SBUF layout per slot: [128, 3, 4, 512]: tile[p, c*2048 + j*512 + w]
holds out[b, c, p*4 + j, w].
"""

from contextlib import ExitStack

import numpy as np

import concourse.bass as bass
import concourse.mybir as mybir
from bass_rust import RegisterHandles, make_scalar_value
from concourse.bass_utils import run_bass_kernel_spmd

B_TOTAL, C, H, W = 64, 3, 512, 512
N_CORES = 8
B = B_TOTAL // N_CORES
MAX_SHIFT = 16
PAD = 2 * MAX_SHIFT  # 32
HP, WP = H + PAD, W + PAD  # 544, 544
CH_ELEMS = HP * WP  # 295936
CH_STRIDE = CH_ELEMS + 64  # 296000, tail pad keeps max window in bounds
P = 128
JH = H // P  # 4 rows of a channel per partition
WIN = P * JH * WP  # 278528: window covering 512 padded rows
MAX_OFF = PAD * WP + PAD  # 17440
NBUF = 6

LOADS_PER_BATCH = C
STORES_PER_BATCH = C


def build_kernel():
    nc = bass.Bass()
    x = nc.dram_tensor("x", [B * C, CH_STRIDE], mybir.dt.float32, kind="ExternalInput")
    shifts = nc.dram_tensor("shifts", [B, 2], mybir.dt.int32, kind="ExternalInput")
    out = nc.dram_tensor("out", [B, C, H, W], mybir.dt.float32, kind="ExternalOutput")

    CW = JH * W  # 2048 elems per channel per partition

    with (
        nc.sbuf_tensor([P, NBUF, C * CW], mybir.dt.float32) as tiles,
        nc.sbuf_tensor([1, B * 2], mybir.dt.int32) as sb_shifts,
        nc.semaphore("pre_sem") as pre_sem,
        ExitStack() as stack,
    ):
        # per (slot, channel) semaphores: stores start as soon as their own
        # 1 MB channel lands (not the whole 3 MB batch), and channel loads
        # only wait for the matching channel's old store -- trims the
        # pipeline ramp and tail by ~2 channels' worth of transfer time.
        load_sems = [
            [
                stack.enter_context(nc.semaphore(f"load_sem{s}_{c}"))
                for c in range(C)
            ]
            for s in range(NBUF)
        ]
        store_sems = [
            [
                stack.enter_context(nc.semaphore(f"store_sem{s}_{c}"))
                for c in range(C)
            ]
            for s in range(NBUF)
        ]
        block = stack.enter_context(nc.Block())

        def emit_loads(eng, my_batches):
            with (
                eng.register("r_off") as r_off,
                eng.register("r_sw") as r_sw,
                eng.register("r_c16") as r_c16,
                eng.register("r_cWP") as r_cWP,
            ):
                eng.reg_mov(r_c16, MAX_SHIFT)
                eng.reg_mov(r_cWP, WP)
                for b in my_batches:
                    s = b % NBUF
                    eng.reg_load(r_off, sb_shifts[0:1, 2 * b : 2 * b + 1])
                    eng.reg_load(r_sw, sb_shifts[0:1, 2 * b + 1 : 2 * b + 2])
                    # r_off = (16 - sh) * 544 + (16 - sw)
                    eng.reg_sub(r_off, r_c16, r_off)
                    eng.reg_sub(r_sw, r_c16, r_sw)
                    eng.reg_mul(r_off, r_off, r_cWP)
                    eng.reg_add(r_off, r_off, r_sw)
                    rb = make_scalar_value(
                        RegisterHandles([r_off]), min_val=0, max_val=MAX_OFF
                    )
                    tile_s = tiles[:, s]
                    for c in range(C):
                        if b >= NBUF:
                            eng.wait_ge(store_sems[s][c], 16 * (b // NBUF))
                        win = x[b * C + c, bass.ds(rb, WIN)]
                        src = win.rearrange("(p j w) -> p j w", j=JH, w=WP)[
                            :, :, 0:W
                        ]
                        eng.dma_start(
                            tile_s[:, c * CW : (c + 1) * CW], src
                        ).then_inc(load_sems[s][c], 16)

        @block.sync
        def _(sync):
            sync.dma_start(
                sb_shifts[0:1, :], shifts.rearrange("b s -> (b s)")[None, :]
            ).then_inc(pre_sem, 16)
            sync.wait_ge(pre_sem, 16)
            emit_loads(sync, list(range(0, B // 2)))

        @block.scalar
        def _(scalar):
            scalar.wait_ge(pre_sem, 16)
            emit_loads(scalar, list(range(B // 2, B)))

        @block.gpsimd
        def _(gp):
            for b in range(B):
                s = b % NBUF
                tile_s = tiles[:, s]
                for c in range(C):
                    gp.wait_ge(load_sems[s][c], 16 * (b // NBUF + 1))
                    gp.dma_start(
                        out[b, c].rearrange("(p j) w -> p (j w)", j=JH),
                        tile_s[:, c * CW : (c + 1) * CW],
                    ).then_inc(store_sems[s][c], 16)
            for s in range(NBUF):
                uses = (B - s + NBUF - 1) // NBUF
                for c in range(C):
                    gp.wait_ge(store_sems[s][c], 16 * uses)

    return nc


_NC_CACHE = None


def _get_nc():
    global _NC_CACHE
    if _NC_CACHE is None:
        _NC_CACHE = build_kernel()
    return _NC_CACHE


def _pad_input(x: np.ndarray) -> np.ndarray:
    """[64, 3, 512, 512] -> [64*3, 296000]: per-channel circular 16-px
    border (544x544) flattened, with 64 tail-pad elems per channel."""
    xp = np.pad(
        x,
        ((0, 0), (0, 0), (MAX_SHIFT, MAX_SHIFT), (MAX_SHIFT, MAX_SHIFT)),
        mode="wrap",
    ).reshape(B_TOTAL * C, CH_ELEMS)
    outp = np.zeros((B_TOTAL * C, CH_STRIDE), dtype=np.float32)
    outp[:, :CH_ELEMS] = xp
    return outp


def kernel(x: np.ndarray, shifts: np.ndarray) -> np.ndarray:
    assert x.shape == (B_TOTAL, C, H, W), x.shape
    assert shifts.shape == (B_TOTAL, 2), shifts.shape
    x = np.ascontiguousarray(x, dtype=np.float32)
    shifts = np.ascontiguousarray(shifts, dtype=np.int32)
    x_pad = _pad_input(x)

    in_maps = [
        {
            "x": x_pad[i * B * C : (i + 1) * B * C],
            "shifts": shifts[i * B : (i + 1) * B],
        }
        for i in range(N_CORES)
    ]
    res = run_bass_kernel_spmd(_get_nc(), in_maps, list(range(N_CORES)))
    return np.concatenate(
        [res.results[i]["out"] for i in range(N_CORES)], axis=0
    ).astype(np.float32)
